# revision 7
# baseline (speedup 1.0000x reference)
"""MultiHeadAttention Trainium2 kernel (8 NeuronCores).

Sharding: core c handles batch b = c // 2 and head-group hg = c % 2
(8 of 16 heads, 512 of 1024 model dims). Attention is embarrassingly
parallel over (b, hg); the output projection is computed per head-group
against the matching W_o columns, yielding partial outputs that the host
sums (plus b_o).

Device dataflow (per core), all in "transposed" layouts so no on-device
transposes are ever needed:
  qT = Wq_hg @ Xq^T      [dh=512, S]   (lhsT = Wq_hg^T, rhs = Xq^T; host preps both)
  kT = Wk_hg @ Xk^T      [dh=512, S]
  v  = Xv @ Wv_hg^T      [S, dh=512]   (+ ones column per head for softmax sums)
  scores_T[k, q] = kT_h[:, kchunk]^T-matmul  (keys on partitions)
  causal mask: extra PE matmul tri^T @ step accumulating -1e9 into masked entries
  probs = exp(scores_T / 8) on ACT (no max subtraction: scores ~ N(0,1), safe)
  attn_T[d, q] (+ sums row) = v_chunk^T-matmul over probs, accumulated in PSUM
  normalize: recip = 1/sums (DVE), broadcast via ones-matmul, multiply (DVE)
  out_partial = attn^T-matmul with Wo columns
"""

import os

import numpy as np

B, S_FULL, D = 4, 2048, 1024
H, DK = 16, 64
NH_G = 8          # heads per core
DH = NH_G * DK    # 512 dims per core
P = 128
KC = 128          # key chunk (PE contraction)
NEG = -1.0e9
SCALE = 1.0 / np.sqrt(np.float32(DK))

_PROG_CACHE = {}


def _dims(S):
    QB = min(512, S)
    return {
        "S": S, "QB": QB, "N_QB": S // QB, "N_KC": S // KC,
        "R": QB // KC, "E_CH": D // P, "M_CH": DH // P, "O_N": D // 512,
    }


def _np_dt(use_bf16):
    if use_bf16:
        import ml_dtypes
        return ml_dtypes.bfloat16
    return np.float32


def build_program(causal, S, use_bf16):
    """Build the single-core Bass/Tile program (same program on all 8 cores)."""
    from contextlib import ExitStack

    import concourse.bass as bass
    import concourse.tile as tile
    from concourse import bacc, mybir

    d = _dims(S)
    QB, N_QB, N_KC, R, E_CH, M_CH, O_N = (
        d["QB"], d["N_QB"], d["N_KC"], d["R"], d["E_CH"], d["M_CH"], d["O_N"])

    DT = mybir.dt.bfloat16 if use_bf16 else mybir.dt.float32r
    F32 = mybir.dt.float32
    F32R = mybir.dt.float32r
    AF = mybir.ActivationFunctionType
    ALU = mybir.AluOpType

    nc = bacc.Bacc("TRN2", target_bir_lowering=False, debug=False)

    xq_t = nc.dram_tensor("xq_t", [D, S], DT, kind="ExternalInput").ap()
    xk_t = nc.dram_tensor("xk_t", [D, S], DT, kind="ExternalInput").ap()
    xv_t = nc.dram_tensor("xv_t", [D, S], DT, kind="ExternalInput").ap()
    wq_t = nc.dram_tensor("wq_t", [D, DH], DT, kind="ExternalInput").ap()
    wk_t = nc.dram_tensor("wk_t", [D, DH], DT, kind="ExternalInput").ap()
    wv_t = nc.dram_tensor("wv_t", [D, DH], DT, kind="ExternalInput").ap()
    wo_t = nc.dram_tensor("wo_t", [DH, D], DT, kind="ExternalInput").ap()
    bq_in = nc.dram_tensor("bq_p", [P, M_CH], F32, kind="ExternalInput").ap()
    bk_in = nc.dram_tensor("bk_p", [P, M_CH], F32, kind="ExternalInput").ap()
    bv_in = nc.dram_tensor("bv_r", [P, DH], F32, kind="ExternalInput").ap()
    tri_in = nc.dram_tensor("tri", [P, KC], DT, kind="ExternalInput").ap()
    stepm_in = nc.dram_tensor("stepm", [P, R, QB], DT, kind="ExternalInput").ap()
    ones_c_in = nc.dram_tensor("ones_c", [65, 64], F32R,
                               kind="ExternalInput").ap()
    ones_v_in = nc.dram_tensor("ones_v", [P, N_KC, NH_G, 1], DT,
                               kind="ExternalInput").ap()
    out_p = nc.dram_tensor("out_p", [S, D], F32, kind="ExternalOutput").ap()

    with tile.TileContext(nc) as tc, ExitStack() as ctx:
        consts = ctx.enter_context(tc.tile_pool(name="consts", bufs=1))
        wpool = ctx.enter_context(tc.tile_pool(name="w", bufs=2))
        qkv = ctx.enter_context(tc.tile_pool(name="qkv", bufs=1))

        tri = consts.tile([P, KC], DT)
        nc.sync.dma_start(tri, tri_in)
        stepm = consts.tile([P, R, QB], DT)
        nc.sync.dma_start(stepm, stepm_in)
        bq_sb = consts.tile([P, M_CH], F32)
        nc.sync.dma_start(bq_sb, bq_in)
        bk_sb = consts.tile([P, M_CH], F32)
        nc.sync.dma_start(bk_sb, bk_in)
        bv_sb = consts.tile([P, DH], F32)
        nc.sync.dma_start(bv_sb, bv_in)
        ones65 = consts.tile([65, 64], F32R)
        nc.sync.dma_start(ones65, ones_c_in)

        qT = qkv.tile([P, M_CH, S], DT, tag="qT")
        kT = qkv.tile([P, M_CH, S], DT, tag="kT")
        v_aug = qkv.tile([P, N_KC, NH_G, 65], DT, tag="v_aug")
        nc.sync.dma_start(v_aug[:, :, :, 64:65], ones_v_in)

        w_tiles = {}
        for name, src in (("wq", wq_t), ("wk", wk_t), ("wv", wv_t)):
            w_sb = wpool.tile([P, E_CH, DH], DT, tag="w")
            nc.sync.dma_start(w_sb, src.rearrange("(eo p) m -> p eo m", p=P))
            w_tiles[name] = w_sb
        wo_sb = wpool.tile([P, M_CH, D], DT, tag="w")
        nc.sync.dma_start(wo_sb, wo_t.rearrange("(mo p) n -> p mo n", p=P))

        # ---- projections ----
        with tc.tile_pool(name="xp", bufs=3) as xpool, \
             tc.tile_pool(name="pj", bufs=3, space="PSUM") as pj_ps:
            for phase, x_in, w_sb, b_sb in (
                ("q", xq_t, w_tiles["wq"], bq_sb),
                ("k", xk_t, w_tiles["wk"], bk_sb),
                ("v", xv_t, w_tiles["wv"], bv_sb),
            ):
                dst = qT if phase == "q" else kT
                for n in range(N_QB):
                    xblk = xpool.tile([P, E_CH, QB], DT, tag="x")
                    nc.sync.dma_start(
                        xblk,
                        x_in.rearrange("(eo p) s -> p eo s", p=P)[
                            :, :, n * QB:(n + 1) * QB],
                    )
                    if phase in ("q", "k"):
                        for m in range(M_CH):
                            ps = pj_ps.tile([P, QB], F32, tag="pj")
                            for e in range(E_CH):
                                nc.tensor.matmul(
                                    ps,
                                    lhsT=w_sb[:, e, m * P:(m + 1) * P],
                                    rhs=xblk[:, e, :],
                                    start=(e == 0), stop=(e == E_CH - 1),
                                )
                            nc.vector.tensor_scalar_add(
                                dst[:, m, n * QB:(n + 1) * QB], ps,
                                b_sb[:, m:m + 1])
                    else:
                        for sc in range(QB // P):
                            ps = pj_ps.tile([P, DH], F32, tag="pj")
                            for e in range(E_CH):
                                nc.tensor.matmul(
                                    ps,
                                    lhsT=xblk[:, e, sc * P:(sc + 1) * P],
                                    rhs=w_sb[:, e, :],
                                    start=(e == 0), stop=(e == E_CH - 1),
                                )
                            kc = n * (QB // P) + sc
                            nc.vector.tensor_tensor(
                                v_aug[:, kc, :, 0:64],
                                ps.rearrange("p (h e) -> p h e", h=NH_G),
                                bv_sb.rearrange("p (h e) -> p h e", h=NH_G),
                                ALU.add,
                            )

        # ---- attention + output projection ----
        with tc.tile_pool(name="sc_ps", bufs=2, space="PSUM") as sc_ps, \
             tc.tile_pool(name="pv_ps", bufs=2, space="PSUM") as pv_pool, \
             tc.tile_pool(name="rb_ps", bufs=1, space="PSUM") as rb_pool, \
             tc.tile_pool(name="op_ps", bufs=1, space="PSUM") as op_ps, \
             tc.tile_pool(name="probs", bufs=4) as probs_pool, \
             tc.tile_pool(name="attn", bufs=M_CH + 1) as attn_pool, \
             tc.tile_pool(name="misc", bufs=3) as misc, \
             tc.tile_pool(name="outst", bufs=3) as outst:
            for qb in range(N_QB):
                attn_tiles = []
                for m in range(M_CH):
                    n_kc = (qb + 1) * (QB // KC) if causal else N_KC
                    pv_t = [pv_pool.tile([65, QB], F32, tag="pv", name=f"pv{hl}")
                             for hl in (0, 1)]
                    for pair in range(n_kc // 2):
                        ps_h = [sc_ps.tile([P, 2 * QB], F32, tag="sc",
                                          name=f"sc{hl}")
                                for hl in (0, 1)]
                        for dkc in (0, 1):
                            kc = 2 * pair + dkc
                            r = kc - (n_kc - R)
                            is_diag = causal and r >= 0
                            for hl in (0, 1):
                                rows = slice(64 * hl, 64 * hl + 64)
                                nc.tensor.matmul(
                                    ps_h[hl][:, dkc * QB:(dkc + 1) * QB],
                                    lhsT=kT[rows, m, kc * KC:(kc + 1) * KC],
                                    rhs=qT[rows, m, qb * QB:(qb + 1) * QB],
                                    start=True, stop=not is_diag,
                                )
                                if is_diag:
                                    nc.tensor.matmul(
                                        ps_h[hl][:, dkc * QB:(dkc + 1) * QB],
                                        lhsT=tri, rhs=stepm[:, r, :],
                                        start=False, stop=True,
                                    )
                        for hl in (0, 1):
                            pt = probs_pool.tile([P, 2 * QB], DT, tag="pt")
                            nc.scalar.activation(pt, ps_h[hl], AF.Exp,
                                                 scale=float(SCALE))
                            for dkc in (0, 1):
                                kc = 2 * pair + dkc
                                nc.tensor.matmul(
                                    pv_t[hl],
                                    lhsT=v_aug[:, kc, 2 * m + hl, :],
                                    rhs=pt[:, dkc * QB:(dkc + 1) * QB],
                                    start=(kc == 0), stop=(kc == n_kc - 1),
                                )
                    attn_m = attn_pool.tile([P, QB], DT, tag="attn")
                    for hl in (0, 1):
                        recip65 = misc.tile([65, QB], F32R, tag="recip")
                        with nc.allow_low_precision(
                                reason="softmax denom reciprocal; f32r"):
                            nc.vector.reciprocal(recip65[64:65, :],
                                                 pv_t[hl][64:65, :])
                        rb = rb_pool.tile([64, QB], F32, tag="rb")
                        nc.tensor.matmul(rb, lhsT=ones65[64:65, :],
                                         rhs=recip65[64:65, :],
                                         start=True, stop=True)
                        attn_u = misc.tile([64, QB], DT, tag="attn_u")
                        nc.any.tensor_copy(attn_u, pv_t[hl][0:64, :])
                        nc.vector.tensor_tensor(
                            attn_m[64 * hl:64 * hl + 64, :], attn_u, rb,
                            ALU.mult)
                    attn_tiles.append(attn_m)
                for ssub in range(QB // P):
                    for nout in range(O_N):
                        pso = op_ps.tile([P, 512], F32, tag="op")
                        for m in range(M_CH):
                            nc.tensor.matmul(
                                pso,
                                lhsT=attn_tiles[m][:, ssub * P:(ssub + 1) * P],
                                rhs=wo_sb[:, m, nout * 512:(nout + 1) * 512],
                                start=(m == 0), stop=(m == M_CH - 1),
                            )
                        st = outst.tile([P, 512], F32, tag="st")
                        nc.any.tensor_copy(st, pso)
                        nc.sync.dma_start(
                            out_p[qb * QB + ssub * P: qb * QB + (ssub + 1) * P,
                                  nout * 512:(nout + 1) * 512],
                            st)
    nc.compile()
    return nc


def make_consts(S, use_bf16):
    """Host-built mask-bias matmul operands (tri, stepm)."""
    d = _dims(S)
    QB, R = d["QB"], d["R"]
    npdt = _np_dt(use_bf16)
    tri = np.zeros((P, KC), np.float32)
    for t in range(P):
        tri[t, t:] = 1.0
    stepm = np.zeros((P, R, QB), np.float32)
    for r in range(R):
        for j in range(QB):
            c = j - KC * r
            if c >= KC - 1:
                continue
            stepm[max(0, c + 1), r, j] = NEG
    return tri.astype(npdt), stepm.astype(npdt)


def core_inputs(Q, K, V, W_q, b_q, W_k, b_k, W_v, b_v, W_o, b, hg, S, use_bf16):
    """Build the per-core input map (host-side slicing/transposition/casts)."""
    npdt = _np_dt(use_bf16)
    d = _dims(S)
    M_CH = d["M_CH"]
    rows = slice(hg * DH, (hg + 1) * DH)

    def t(x):
        return np.ascontiguousarray(np.asarray(x, np.float32).T).astype(npdt)

    tri, stepm = make_consts(S, use_bf16)
    return {
        "xq_t": t(Q[b]), "xk_t": t(K[b]), "xv_t": t(V[b]),
        "wq_t": t(W_q[rows]), "wk_t": t(W_k[rows]), "wv_t": t(W_v[rows]),
        "wo_t": t(W_o[:, rows]),
        "bq_p": np.ascontiguousarray(
            np.asarray(b_q[rows], np.float32).reshape(M_CH, P).T),
        "bk_p": np.ascontiguousarray(
            np.asarray(b_k[rows], np.float32).reshape(M_CH, P).T),
        "bv_r": np.broadcast_to(
            np.asarray(b_v[rows], np.float32), (P, DH)).copy(),
        "tri": tri, "stepm": stepm,
        "ones_c": np.ones((65, 64), np.float32),
        "ones_v": np.ones((P, d["N_KC"], NH_G, 1), npdt),
    }


def _np_reference(Q, K, V, mask, W_q, b_q, W_k, b_k, W_v, b_v, W_o, b_o):
    """Exact numpy fallback for arbitrary masks."""
    q = (Q @ W_q.T + b_q).reshape(B, S_FULL, H, DK).transpose(0, 2, 1, 3)
    k = (K @ W_k.T + b_k).reshape(B, S_FULL, H, DK).transpose(0, 2, 1, 3)
    v = (V @ W_v.T + b_v).reshape(B, S_FULL, H, DK).transpose(0, 2, 1, 3)
    scores = np.einsum("bhqd,bhkd->bhqk", q, k) / np.sqrt(np.float32(DK))
    scores = np.where(mask == 0, np.finfo(np.float32).min, scores)
    scores -= scores.max(-1, keepdims=True)
    probs = np.exp(scores)
    probs /= probs.sum(-1, keepdims=True)
    out = np.einsum("bhqk,bhkd->bhqd", probs, v)
    out = out.transpose(0, 2, 1, 3).reshape(B, S_FULL, D)
    return (out @ W_o.T + b_o).astype(np.float32)


def kernel(Q, K, V, mask, W_q, b_q, W_k, b_k, W_v, b_v, W_o, b_o):
    Q = np.asarray(Q, np.float32)
    K = np.asarray(K, np.float32)
    V = np.asarray(V, np.float32)
    mask = np.asarray(mask)

    m2 = mask.reshape(mask.shape[-2], mask.shape[-1])
    if np.array_equal(m2 != 0, np.tril(np.ones(m2.shape, bool))):
        causal = True
    elif (m2 != 0).all():
        causal = False
    else:
        return _np_reference(Q, K, V, mask, W_q, b_q, W_k, b_k, W_v, b_v,
                             W_o, b_o)

    use_bf16 = os.environ.get("MHA_KERNEL_DTYPE", "f32r") == "bf16"
    from concourse.bass_utils import run_bass_kernel_spmd

    key = (causal, S_FULL, use_bf16)
    if key not in _PROG_CACHE:
        _PROG_CACHE[key] = build_program(causal, S_FULL, use_bf16)
    nc = _PROG_CACHE[key]

    in_maps = []
    for c in range(8):
        b, hg = divmod(c, 2)
        in_maps.append(core_inputs(Q, K, V, W_q, b_q, W_k, b_k, W_v, b_v,
                                   W_o, b, hg, S_FULL, use_bf16))

    trace = os.environ.get("MHA_KERNEL_TRACE", "0") == "1"
    kw = {}
    if trace:
        kw = {"trace": True,
              "trace_cores": [int(x) for x in os.environ.get(
                  "MHA_TRACE_CORES", "0").split(",")]}
    res = run_bass_kernel_spmd(nc, in_maps, core_ids=list(range(8)), **kw)
    kernel.last_results = res

    b_o32 = np.asarray(b_o, np.float32)
    out = np.empty((B, S_FULL, D), np.float32)
    for b in range(B):
        out[b] = (res.results[2 * b]["out_p"] + res.results[2 * b + 1]["out_p"]
                  + b_o32[None, :])
    return out


kernel.last_results = None


# revision 13
# speedup vs baseline: 1.4115x; 1.4115x over previous
"""MultiHeadAttention Trainium2 kernel (8 NeuronCores).

Sharding: core c handles batch b = c // 2 and head-group hg = c % 2
(8 of 16 heads, 512 of 1024 model dims). Attention is embarrassingly
parallel over (b, hg); the output projection is computed per head-group
against the matching W_o columns, yielding partial outputs that the host
sums (plus b_o).

Device dataflow (per core), all in "transposed" layouts so no on-device
transposes are ever needed:
  qT = Wq_hg @ Xq^T      [dh=512, S]   (lhsT = Wq_hg^T, rhs = Xq^T; host preps both)
  kT = Wk_hg @ Xk^T      [dh=512, S]
  v  = Xv @ Wv_hg^T      [S, dh=512]   (+ ones column per head for softmax sums)
  scores_T[k, q] = kT_h[:, kchunk]^T-matmul  (keys on partitions)
  causal mask: extra PE matmul tri^T @ step accumulating -1e9 into masked entries
  probs = exp(scores_T / 8) on ACT (no max subtraction: scores ~ N(0,1), safe)
  attn_T[d, q] (+ sums row) = v_chunk^T-matmul over probs, accumulated in PSUM
  normalize: recip = 1/sums (DVE), broadcast via ones-matmul, multiply (DVE)
  out_partial = attn^T-matmul with Wo columns
"""

import os

import numpy as np

B, S_FULL, D = 4, 2048, 1024
H, DK = 16, 64
NH_G = 8          # heads per core
DH = NH_G * DK    # 512 dims per core
P = 128
KC = 128          # key chunk (PE contraction)
NEG = -1.0e9
SCALE = 1.0 / np.sqrt(np.float32(DK))

_PROG_CACHE = {}


def _dims(S):
    QB = min(512, S)
    return {
        "S": S, "QB": QB, "N_QB": S // QB, "N_KC": S // KC,
        "R": QB // KC, "E_CH": D // P, "M_CH": DH // P, "O_N": D // 512,
    }


def _np_dt(use_bf16):
    if use_bf16:
        import ml_dtypes
        return ml_dtypes.bfloat16
    return np.float32


def build_program(causal, S, use_bf16, debug_dumps=False):
    """Build the single-core Bass/Tile program (same program on all 8 cores)."""
    from contextlib import ExitStack

    import concourse.bass as bass
    import concourse.tile as tile
    from concourse import bacc, mybir

    d = _dims(S)
    QB, N_QB, N_KC, R, E_CH, M_CH, O_N = (
        d["QB"], d["N_QB"], d["N_KC"], d["R"], d["E_CH"], d["M_CH"], d["O_N"])

    DT = mybir.dt.bfloat16 if use_bf16 else mybir.dt.float32r
    F32 = mybir.dt.float32
    F32R = mybir.dt.float32r
    AF = mybir.ActivationFunctionType
    ALU = mybir.AluOpType

    nc = bacc.Bacc("TRN2", target_bir_lowering=False, debug=False)

    xq_t = nc.dram_tensor("xq_t", [D, S], DT, kind="ExternalInput").ap()
    xk_t = nc.dram_tensor("xk_t", [D, S], DT, kind="ExternalInput").ap()
    xv_t = nc.dram_tensor("xv_t", [D, S], DT, kind="ExternalInput").ap()
    wq_t = nc.dram_tensor("wq_t", [D, DH], DT, kind="ExternalInput").ap()
    wk_t = nc.dram_tensor("wk_t", [D, DH], DT, kind="ExternalInput").ap()
    wv_t = nc.dram_tensor("wv_t", [D, DH], DT, kind="ExternalInput").ap()
    wo_t = nc.dram_tensor("wo_t", [DH, D], DT, kind="ExternalInput").ap()
    bq_in = nc.dram_tensor("bq_p", [P, M_CH], F32, kind="ExternalInput").ap()
    bk_in = nc.dram_tensor("bk_p", [P, M_CH], F32, kind="ExternalInput").ap()
    bv_in = nc.dram_tensor("bv_r", [P, DH], F32, kind="ExternalInput").ap()
    tri_in = nc.dram_tensor("tri", [P, KC], DT, kind="ExternalInput").ap()
    stepm_in = nc.dram_tensor("stepm", [P, R, QB], DT, kind="ExternalInput").ap()
    ones_c_in = nc.dram_tensor("ones_c", [65, 64], F32R,
                               kind="ExternalInput").ap()
    ones_v_in = nc.dram_tensor("ones_v", [P, N_KC, NH_G, 1], DT,
                               kind="ExternalInput").ap()
    out_p = nc.dram_tensor("out_p", [S, D], F32, kind="ExternalOutput").ap()
    if debug_dumps:
        dbg_qT = nc.dram_tensor("dbg_qT", [P, M_CH, S], DT,
                                kind="ExternalOutput").ap()
        dbg_kT = nc.dram_tensor("dbg_kT", [P, M_CH, S], DT,
                                kind="ExternalOutput").ap()
        dbg_vaug = nc.dram_tensor("dbg_vaug", [P, N_KC, NH_G, 65], DT,
                                  kind="ExternalOutput").ap()
        dbg_probs = nc.dram_tensor("dbg_probs", [P, 2 * QB], DT,
                                   kind="ExternalOutput").ap()
        dbg_attn = nc.dram_tensor("dbg_attn", [M_CH, P, QB], DT,
                                  kind="ExternalOutput").ap()
        dbg_recip = nc.dram_tensor("dbg_recip", [P, 3, QB], F32,
                                   kind="ExternalOutput").ap()

    with tile.TileContext(nc) as tc, ExitStack() as ctx:
        consts = ctx.enter_context(tc.tile_pool(name="consts", bufs=1))
        wpool = ctx.enter_context(tc.tile_pool(name="w", bufs=2))
        qkv = ctx.enter_context(tc.tile_pool(name="qkv", bufs=1))

        tri = consts.tile([P, KC], DT)
        nc.sync.dma_start(tri, tri_in)
        stepm = consts.tile([P, R, QB], DT)
        nc.sync.dma_start(stepm, stepm_in)
        bq_sb = consts.tile([P, M_CH], F32)
        nc.sync.dma_start(bq_sb, bq_in)
        bk_sb = consts.tile([P, M_CH], F32)
        nc.sync.dma_start(bk_sb, bk_in)
        bv_sb = consts.tile([P, DH], F32)
        nc.sync.dma_start(bv_sb, bv_in)
        ones65 = consts.tile([65, 64], F32R)
        nc.sync.dma_start(ones65, ones_c_in)

        qT = qkv.tile([P, M_CH, S], DT, tag="qT")
        kT = qkv.tile([P, M_CH, S], DT, tag="kT")
        v_aug = qkv.tile([P, N_KC, NH_G, 65], DT, tag="v_aug")
        nc.sync.dma_start(v_aug[:, :, :, 64:65], ones_v_in)

        w_tiles = {}
        for name, src in (("wq", wq_t), ("wk", wk_t), ("wv", wv_t)):
            w_sb = wpool.tile([P, E_CH, DH], DT, tag="w")
            nc.sync.dma_start(w_sb, src.rearrange("(eo p) m -> p eo m", p=P))
            w_tiles[name] = w_sb
        wo_sb = wpool.tile([P, M_CH, D], DT, tag="w")
        nc.sync.dma_start(wo_sb, wo_t.rearrange("(mo p) n -> p mo n", p=P))

        # ---- projections ----
        with tc.tile_pool(name="xp", bufs=3) as xpool, \
             tc.tile_pool(name="pj", bufs=3, space="PSUM") as pj_ps:
            for phase, x_in, w_sb, b_sb in (
                ("q", xq_t, w_tiles["wq"], bq_sb),
                ("k", xk_t, w_tiles["wk"], bk_sb),
                ("v", xv_t, w_tiles["wv"], bv_sb),
            ):
                dst = qT if phase == "q" else kT
                for n in range(N_QB):
                    xblk = xpool.tile([P, E_CH, QB], DT, tag="x")
                    nc.sync.dma_start(
                        xblk,
                        x_in.rearrange("(eo p) s -> p eo s", p=P)[
                            :, :, n * QB:(n + 1) * QB],
                    )
                    if phase in ("q", "k"):
                        for m in range(M_CH):
                            ps = pj_ps.tile([P, QB], F32, tag="pj")
                            for e in range(E_CH):
                                nc.tensor.matmul(
                                    ps,
                                    lhsT=w_sb[:, e, m * P:(m + 1) * P],
                                    rhs=xblk[:, e, :],
                                    start=(e == 0), stop=(e == E_CH - 1),
                                )
                            nc.vector.tensor_scalar_add(
                                dst[:, m, n * QB:(n + 1) * QB], ps,
                                b_sb[:, m:m + 1])
                    else:
                        for sc in range(QB // P):
                            ps = pj_ps.tile([P, DH], F32, tag="pj")
                            for e in range(E_CH):
                                nc.tensor.matmul(
                                    ps,
                                    lhsT=xblk[:, e, sc * P:(sc + 1) * P],
                                    rhs=w_sb[:, e, :],
                                    start=(e == 0), stop=(e == E_CH - 1),
                                )
                            kc = n * (QB // P) + sc
                            nc.vector.tensor_tensor(
                                v_aug[:, kc, :, 0:64],
                                ps.rearrange("p (h e) -> p h e", h=NH_G),
                                bv_sb.rearrange("p (h e) -> p h e", h=NH_G),
                                ALU.add,
                            )

        if debug_dumps:
            nc.sync.dma_start(dbg_qT, qT)
            nc.sync.dma_start(dbg_kT, kT)
            nc.sync.dma_start(dbg_vaug, v_aug)

        # ---- attention + output projection ----
        with tc.tile_pool(name="sc_ps", bufs=2, space="PSUM") as sc_ps, \
             tc.tile_pool(name="pv_ps", bufs=2, space="PSUM") as pv_pool, \
             tc.tile_pool(name="rb_ps", bufs=1, space="PSUM") as rb_pool, \
             tc.tile_pool(name="op_ps", bufs=1, space="PSUM") as op_ps, \
             tc.tile_pool(name="probs", bufs=4) as probs_pool, \
             tc.tile_pool(name="attn", bufs=M_CH + 1) as attn_pool, \
             tc.tile_pool(name="misc", bufs=3) as misc, \
             tc.tile_pool(name="outst", bufs=3) as outst:
            for qb in range(N_QB):
                attn_tiles = []
                for m in range(M_CH):
                    n_kc = (qb + 1) * (QB // KC) if causal else N_KC
                    pv_t = [pv_pool.tile([65, QB], F32, tag="pv", name=f"pv{hl}")
                             for hl in (0, 1)]
                    for pair in range(n_kc // 2):
                        ps_h = [sc_ps.tile([P, 2 * QB], F32, tag="sc",
                                          name=f"sc{hl}")
                                for hl in (0, 1)]
                        for dkc in (0, 1):
                            kc = 2 * pair + dkc
                            r = kc - (n_kc - R)
                            is_diag = causal and r >= 0
                            for hl in (0, 1):
                                rows = slice(64 * hl, 64 * hl + 64)
                                nc.tensor.matmul(
                                    ps_h[hl][:, dkc * QB:(dkc + 1) * QB],
                                    lhsT=kT[rows, m, kc * KC:(kc + 1) * KC],
                                    rhs=qT[rows, m, qb * QB:(qb + 1) * QB],
                                    start=True, stop=not is_diag,
                                )
                                if is_diag:
                                    nc.tensor.matmul(
                                        ps_h[hl][:, dkc * QB:(dkc + 1) * QB],
                                        lhsT=tri, rhs=stepm[:, r, :],
                                        start=False, stop=True,
                                    )
                        for hl in (0, 1):
                            pt = probs_pool.tile([P, 2 * QB], DT, tag="pt")
                            nc.scalar.activation(pt, ps_h[hl], AF.Exp,
                                                 scale=float(SCALE))
                            if (debug_dumps and qb == 0 and m == 0
                                    and pair == 0 and hl == 0):
                                nc.sync.dma_start(dbg_probs, pt)
                            for dkc in (0, 1):
                                kc = 2 * pair + dkc
                                nc.tensor.matmul(
                                    pv_t[hl],
                                    lhsT=v_aug[:, kc, 2 * m + hl, :],
                                    rhs=pt[:, dkc * QB:(dkc + 1) * QB],
                                    start=(kc == 0), stop=(kc == n_kc - 1),
                                )
                    attn_m = attn_pool.tile([P, QB], DT, tag="attn")
                    for hl in (0, 1):
                        recip65 = misc.tile([65, QB], F32R, tag="recip")
                        with nc.allow_low_precision(
                                reason="softmax denom recip, f32r rounding"):
                            nc.vector.reciprocal(recip65[64:65, :],
                                                 pv_t[hl][64:65, :])
                        rb = rb_pool.tile([64, QB], F32, tag="rb")
                        nc.tensor.matmul(rb, lhsT=ones65[64:65, :],
                                         rhs=recip65[64:65, :],
                                         start=True, stop=True)
                        attn_u = misc.tile([64, QB], DT, tag="attn_u")
                        nc.any.tensor_copy(attn_u, pv_t[hl][0:64, :])
                        if debug_dumps and qb == 0 and m == 0 and hl == 0:
                            nc.sync.dma_start(
                                dbg_recip[64:65, 1, :],
                                recip65[64:65, :].bitcast(F32))
                            nc.sync.dma_start(
                                dbg_recip[64:65, 0, :],
                                recip65[64:65, :].bitcast(F32))
                            rbsb = misc.tile([64, QB], F32, tag="rbsb",
                                             name="rbsb")
                            nc.vector.tensor_copy(rbsb, rb)
                            nc.sync.dma_start(dbg_recip[0:64, 2, :], rbsb)
                        nc.vector.tensor_tensor(
                            attn_m[64 * hl:64 * hl + 64, :], attn_u, rb,
                            ALU.mult)
                    if debug_dumps and qb == 0:
                        nc.sync.dma_start(dbg_attn[m], attn_m)
                    attn_tiles.append(attn_m)
                for ssub in range(QB // P):
                    for nout in range(O_N):
                        pso = op_ps.tile([P, 512], F32, tag="op")
                        for m in range(M_CH):
                            nc.tensor.matmul(
                                pso,
                                lhsT=attn_tiles[m][:, ssub * P:(ssub + 1) * P],
                                rhs=wo_sb[:, m, nout * 512:(nout + 1) * 512],
                                start=(m == 0), stop=(m == M_CH - 1),
                            )
                        st = outst.tile([P, 512], F32, tag="st")
                        nc.any.tensor_copy(st, pso)
                        nc.sync.dma_start(
                            out_p[qb * QB + ssub * P: qb * QB + (ssub + 1) * P,
                                  nout * 512:(nout + 1) * 512],
                            st)
    nc.compile()
    return nc


def make_consts(S, use_bf16):
    """Host-built mask-bias matmul operands (tri, stepm)."""
    d = _dims(S)
    QB, R = d["QB"], d["R"]
    npdt = _np_dt(use_bf16)
    tri = np.zeros((P, KC), np.float32)
    for t in range(P):
        tri[t, t:] = 1.0
    stepm = np.zeros((P, R, QB), np.float32)
    for r in range(R):
        for j in range(QB):
            c = j - KC * r
            if c >= KC - 1:
                continue
            stepm[max(0, c + 1), r, j] = NEG
    return tri.astype(npdt), stepm.astype(npdt)


def core_inputs(Q, K, V, W_q, b_q, W_k, b_k, W_v, b_v, W_o, b, hg, S, use_bf16):
    """Build the per-core input map (host-side slicing/transposition/casts)."""
    npdt = _np_dt(use_bf16)
    d = _dims(S)
    M_CH = d["M_CH"]
    rows = slice(hg * DH, (hg + 1) * DH)

    def t(x):
        return np.ascontiguousarray(np.asarray(x, np.float32).T).astype(npdt)

    tri, stepm = make_consts(S, use_bf16)
    return {
        "xq_t": t(Q[b]), "xk_t": t(K[b]), "xv_t": t(V[b]),
        "wq_t": t(W_q[rows]), "wk_t": t(W_k[rows]), "wv_t": t(W_v[rows]),
        "wo_t": t(W_o[:, rows]),
        "bq_p": np.ascontiguousarray(
            np.asarray(b_q[rows], np.float32).reshape(M_CH, P).T),
        "bk_p": np.ascontiguousarray(
            np.asarray(b_k[rows], np.float32).reshape(M_CH, P).T),
        "bv_r": np.broadcast_to(
            np.asarray(b_v[rows], np.float32), (P, DH)).copy(),
        "tri": tri, "stepm": stepm,
        "ones_c": np.ones((65, 64), np.float32),
        "ones_v": np.ones((P, d["N_KC"], NH_G, 1), npdt),
    }


def _np_reference(Q, K, V, mask, W_q, b_q, W_k, b_k, W_v, b_v, W_o, b_o):
    """Exact numpy fallback for arbitrary masks."""
    q = (Q @ W_q.T + b_q).reshape(B, S_FULL, H, DK).transpose(0, 2, 1, 3)
    k = (K @ W_k.T + b_k).reshape(B, S_FULL, H, DK).transpose(0, 2, 1, 3)
    v = (V @ W_v.T + b_v).reshape(B, S_FULL, H, DK).transpose(0, 2, 1, 3)
    scores = np.einsum("bhqd,bhkd->bhqk", q, k) / np.sqrt(np.float32(DK))
    scores = np.where(mask == 0, np.finfo(np.float32).min, scores)
    scores -= scores.max(-1, keepdims=True)
    probs = np.exp(scores)
    probs /= probs.sum(-1, keepdims=True)
    out = np.einsum("bhqk,bhkd->bhqd", probs, v)
    out = out.transpose(0, 2, 1, 3).reshape(B, S_FULL, D)
    return (out @ W_o.T + b_o).astype(np.float32)


def kernel(Q, K, V, mask, W_q, b_q, W_k, b_k, W_v, b_v, W_o, b_o):
    Q = np.asarray(Q, np.float32)
    K = np.asarray(K, np.float32)
    V = np.asarray(V, np.float32)
    mask = np.asarray(mask)

    m2 = mask.reshape(mask.shape[-2], mask.shape[-1])
    if np.array_equal(m2 != 0, np.tril(np.ones(m2.shape, bool))):
        causal = True
    elif (m2 != 0).all():
        causal = False
    else:
        return _np_reference(Q, K, V, mask, W_q, b_q, W_k, b_k, W_v, b_v,
                             W_o, b_o)

    use_bf16 = os.environ.get("MHA_KERNEL_DTYPE", "f32r") == "bf16"
    from concourse.bass_utils import run_bass_kernel_spmd

    key = (causal, S_FULL, use_bf16)
    if key not in _PROG_CACHE:
        _PROG_CACHE[key] = build_program(causal, S_FULL, use_bf16)
    nc = _PROG_CACHE[key]

    in_maps = []
    for c in range(8):
        b, hg = divmod(c, 2)
        in_maps.append(core_inputs(Q, K, V, W_q, b_q, W_k, b_k, W_v, b_v,
                                   W_o, b, hg, S_FULL, use_bf16))

    trace = os.environ.get("MHA_KERNEL_TRACE", "0") == "1"
    kw = {}
    if trace:
        kw = {"trace": True,
              "trace_cores": [int(x) for x in os.environ.get(
                  "MHA_TRACE_CORES", "0").split(",")]}
    res = run_bass_kernel_spmd(nc, in_maps, core_ids=list(range(8)), **kw)
    kernel.last_results = res

    b_o32 = np.asarray(b_o, np.float32)
    out = np.empty((B, S_FULL, D), np.float32)
    for b in range(B):
        out[b] = (res.results[2 * b]["out_p"] + res.results[2 * b + 1]["out_p"]
                  + b_o32[None, :])
    return out


kernel.last_results = None
